# revision 2
# baseline (speedup 1.0000x reference)
"""Trainium2 Bass kernel for nn_Attn_76424648065726.

Computes softmax(einsum('so,o->s', outputs @ W.T + b, w)) reshaped to
[1, 1, S].

Math: (outputs @ W.T + b) @ w == outputs @ (W.T @ w) + dot(b, w), and the
scalar dot(b, w) cancels inside softmax.  So the kernel computes
softmax(outputs @ v) with v = W.T @ w — turning the [S,H2]x[H2,H2] matmul
into a memory-bound matvec pipeline.

Sharding (8 cores, hidden-dim parallel): core k owns columns
[512k, 512k+512) of both W and outputs.
  phase 1: v_k = W[:, cols_k].T @ w                  (PE, PSUM accumulate)
  phase 2: e_k[s] = outputs[s, cols_k] @ v_k         (DVE tensor_tensor_reduce)
  AllReduce(add) over e_k -> full energies on every core
  softmax on-device (redundant per core), host takes core 0's output.
"""

import numpy as np

N_CORES = 8
S = 8192
H2 = 4096
HS = H2 // N_CORES  # 512 columns of W / outputs per core
N_OCHUNK = H2 // 128  # 32 contraction chunks for v
N_SBLK = S // 128  # 64 row-blocks of outputs per core

_CACHE = {}


def _build_nc(enable_asserts=False):
    import concourse.bass as bass
    import concourse.tile as tile
    from concourse import bacc, mybir

    nc = bacc.Bacc(
        "TRN2",
        target_bir_lowering=False,
        debug=False,
        enable_asserts=enable_asserts,
        num_devices=N_CORES,
    )
    fp32 = mybir.dt.float32
    x_d = nc.dram_tensor("x", [S, HS], fp32, kind="ExternalInput").ap()
    wc_d = nc.dram_tensor("wc", [H2, HS], fp32, kind="ExternalInput").ap()
    wt_d = nc.dram_tensor("wt", [128, N_OCHUNK], fp32, kind="ExternalInput").ap()
    p_d = nc.dram_tensor("p", [128, N_SBLK], fp32, kind="ExternalOutput").ap()

    with tile.TileContext(nc) as tc:
        _body(tc, x_d, wc_d, wt_d, p_d)
    nc.compile()
    return nc


def _body(tc, x_d, wc_d, wt_d, p_d):
    import concourse.bass as bass
    from concourse import bass_isa, mybir

    nc = tc.nc
    fp32 = mybir.dt.float32
    ts = bass.ts

    from contextlib import ExitStack

    with ExitStack() as ctx:
        wpool = ctx.enter_context(tc.tile_pool(name="wpool", bufs=4))
        xpool = ctx.enter_context(tc.tile_pool(name="xpool", bufs=12))
        spsum = ctx.enter_context(tc.tile_pool(name="spsum", bufs=4, space="PSUM"))
        vpsum = ctx.enter_context(tc.tile_pool(name="vpsum", bufs=1, space="PSUM"))
        small = ctx.enter_context(tc.tile_pool(name="small", bufs=1))
        dram = ctx.enter_context(tc.tile_pool(name="dram", bufs=1, space="DRAM"))

        # w, pre-transposed on host to [128, 32]: wt[p, c] = w[c*128 + p]
        wt_sb = small.tile([128, N_OCHUNK], fp32)
        nc.sync.dma_start(wt_sb[:], wt_d[:])

        # ---- phase 1: v = W_k.T @ w  ([1, HS] accumulated in PSUM) ----
        v_ps = vpsum.tile([1, HS], fp32)
        for c in range(N_OCHUNK):
            wtile = wpool.tile([128, HS], fp32)
            nc.sync.dma_start(wtile[:], wc_d[ts(c, 128), :])
            nc.tensor.matmul(
                v_ps[:],
                lhsT=wt_sb[:, c : c + 1],
                rhs=wtile[:],
                start=(c == 0),
                stop=(c == N_OCHUNK - 1),
            )

        v_row = small.tile([1, HS], fp32)
        nc.scalar.copy(v_row[:], v_ps[:])
        vb = small.tile([128, HS], fp32)
        nc.gpsimd.partition_broadcast(vb[:], v_row[:])

        # ---- phase 2: partial energies e_sb[p, b] = X[128b+p, :] @ v_k ----
        # scalar_tensor_tensor: out = (in0 * 1.0) * in1, accum_out = rowsum
        # (tensor_tensor_reduce dies with INTERNAL on this HW path)
        e_sb = small.tile([128, N_SBLK], fp32)
        for b in range(N_SBLK):
            xt = xpool.tile([128, HS], fp32)
            nc.scalar.dma_start(xt[:], x_d[ts(b, 128), :])
            scr = spsum.tile([128, HS], fp32)
            nc.vector.scalar_tensor_tensor(
                out=scr[:],
                in0=xt[:],
                scalar=1.0,
                in1=vb[:],
                op0=mybir.AluOpType.mult,
                op1=mybir.AluOpType.mult,
                accum_out=e_sb[:, b : b + 1],
            )

        # ---- AllReduce partial energies across the 8 cores ----
        e_dr = dram.tile([128, N_SBLK], fp32)
        e_red = dram.tile([128, N_SBLK], fp32)
        nc.sync.dma_start(e_dr[:], e_sb[:])
        nc.gpsimd.collective_compute(
            "AllReduce",
            mybir.AluOpType.add,
            replica_groups=[list(range(N_CORES))],
            ins=[e_dr.opt()],
            outs=[e_red.opt()],
        )
        ef = small.tile([128, N_SBLK], fp32)
        nc.sync.dma_start(ef[:], e_red[:])

        # ---- softmax over all S values (redundant on every core) ----
        m1 = small.tile([128, 1], fp32)
        nc.vector.tensor_reduce(
            m1[:], ef[:], axis=mybir.AxisListType.X, op=mybir.AluOpType.max
        )
        mb = small.tile([128, 1], fp32)
        nc.gpsimd.partition_all_reduce(
            mb[:], m1[:], channels=128, reduce_op=bass_isa.ReduceOp.max
        )
        nmb = small.tile([128, 1], fp32)
        nc.scalar.mul(nmb[:], mb[:], -1.0)
        pexp = small.tile([128, N_SBLK], fp32)
        s1 = small.tile([128, 1], fp32)
        nc.scalar.activation(
            pexp[:],
            ef[:],
            mybir.ActivationFunctionType.Exp,
            bias=nmb[:],
            scale=1.0,
            accum_out=s1[:],
        )
        zb = small.tile([128, 1], fp32)
        nc.gpsimd.partition_all_reduce(
            zb[:], s1[:], channels=128, reduce_op=bass_isa.ReduceOp.add
        )
        rz = small.tile([128, 1], fp32)
        nc.vector.reciprocal(rz[:], zb[:])
        po = small.tile([128, N_SBLK], fp32)
        nc.scalar.mul(po[:], pexp[:], rz[:])
        nc.sync.dma_start(p_d[:], po[:])


def _shard_inputs(outputs, W, w):
    outputs = np.ascontiguousarray(np.asarray(outputs, dtype=np.float32))
    W = np.ascontiguousarray(np.asarray(W, dtype=np.float32))
    w = np.ascontiguousarray(np.asarray(w, dtype=np.float32))
    wt = np.ascontiguousarray(w.reshape(N_OCHUNK, 128).T)
    in_maps = []
    for k in range(N_CORES):
        cols = slice(HS * k, HS * (k + 1))
        in_maps.append(
            {
                "x": np.ascontiguousarray(outputs[:, cols]),
                "wc": np.ascontiguousarray(W[:, cols]),
                "wt": wt,
            }
        )
    return in_maps


def _run(outputs, W, w, trace=False):
    from concourse.bass_utils import run_bass_kernel_spmd

    if "nc" not in _CACHE:
        _CACHE["nc"] = _build_nc()
    nc = _CACHE["nc"]
    in_maps = _shard_inputs(outputs, W, w)
    res = run_bass_kernel_spmd(
        nc, in_maps, list(range(N_CORES)), trace=trace
    )
    p = res.results[0]["p"]  # [128, 64]; full[s = c*128 + p] = p[p, c]
    full = np.ascontiguousarray(p.T).reshape(1, 1, S).astype(np.float32)
    return full, res


def kernel(outputs, W, b, w):
    out, _ = _run(outputs, W, w, trace=False)
    return out


def kernel_traced(outputs, W, b, w):
    out, res = _run(outputs, W, w, trace=True)
    return out, res
